# revision 15
# baseline (speedup 1.0000x reference)
"""Trainium2 Bass kernel for nn_Conv2d_85830626443584.

Math (from the reference):
  x: [16, 64, 128, 128] f32, W: [8, 9] f32
  s = silu(x)
  out[b, c*8+k, ho, wo] = sum_{dh,dw} W[k, 3*dh+dw] * s[b, c, ho+dh, wo+dw]
  out: [16, 512, 126, 126] f32

Strategy (per NeuronCore, batch-sharded 16/8 = 2 batches -> 128 channel-images):
  * Each channel-image is an independent [128, 128] tile, SBUF layout
    [partition=h, free=w].
  * The 3x3 conv is computed as 3 PSUM-accumulating matmuls per output map k:
    a banded stationary matrix Band[(h_in=128), (ho=126)] carries the 3
    vertical taps (dh), and the horizontal taps (dw) come for free as
    rhs access-pattern column offsets:
       psum_k[ho, n] += sum_h Band_{k,dw}[h, ho] * s[h, n+dw]   (dw = 0,1,2)
    No im2col, no data duplication.
  * bf16 band + bf16 silu: matmul streams 1 col/cycle and the bf16 weight
    load (vs 4-byte f32r) hides under the 504-col stream.
  * Output is stored in bf16 in a DMA-friendly device layout
    [group, ho, (img, k, wo)] -- per-partition contiguous 8KB descriptors
    instead of 504B rows -- and reassembled + upcast to f32 on the host.
  * Images are processed in groups of 4 (rhs N = 4*126 = 504 <= 512 psum bank).
"""

import numpy as np

B, C, H, WD = 16, 64, 128, 128
NK = 8            # n_convs
HO = WO = 126     # output spatial dims
NCORES = 8
B_LOC = B // NCORES              # 2 batches per core
NIMG_LOC = B_LOC * C             # 128 images per core
GRP = 4                          # images per group
NGRP = NIMG_LOC // GRP           # 32 groups
FREE = GRP * NK * WO             # 4032 output elems per partition per group

_CACHE = {}


def _make_bands(W: np.ndarray) -> np.ndarray:
    """Banded stationary matrices, one [128, 126] per (k, dw).

    bands[h, k, dw, ho] = W[k, 3*dh + dw] where dh = h - ho in {0,1,2}.
    Returned flattened to [128, 8*3*126], float32 (caller casts to bf16).
    """
    bands = np.zeros((H, NK, 3, HO), dtype=np.float32)
    ho = np.arange(HO)
    for dh in range(3):
        for dw in range(3):
            bands[ho + dh, :, dw, ho] = W[:, 3 * dh + dw][None, :]
    return bands.reshape(H, NK * 3 * HO)


def _build_module(native_silu: bool = True):
    import concourse.mybir as mybir
    import concourse.tile as tile
    from concourse import bacc
    from contextlib import ExitStack

    f32 = mybir.dt.float32
    bf16 = mybir.dt.bfloat16

    nc = bacc.Bacc("TRN2", target_bir_lowering=False, debug=False)

    x_d = nc.dram_tensor("x", [B_LOC, C, H, WD], f32, kind="ExternalInput")
    bands_d = nc.dram_tensor("bands", [H, NK * 3 * HO], bf16, kind="ExternalInput")
    out_d = nc.dram_tensor("out", [NGRP, HO, FREE], bf16, kind="ExternalOutput")

    with tile.TileContext(nc) as tc, ExitStack() as ctx:
        cpool = ctx.enter_context(tc.tile_pool(name="const", bufs=1))
        xpool = ctx.enter_context(tc.tile_pool(name="xin", bufs=3))
        spool = ctx.enter_context(tc.tile_pool(name="silu", bufs=3))
        opool = ctx.enter_context(tc.tile_pool(name="outs", bufs=3))
        ppool = ctx.enter_context(tc.tile_pool(name="psum", bufs=8, space="PSUM"))

        band_t = cpool.tile([H, NK * 3 * HO], bf16)
        # band on scalar's queue so sync can issue group-0's load at once
        nc.scalar.dma_start(band_t[:], bands_d.ap())
        band4 = band_t[:].rearrange("p (k d m) -> p k d m", k=NK, d=3)

        # [128 images, 128 h, 128 w] view of the local input
        x_flat = x_d.ap().rearrange("b c h w -> (b c) h w")

        def load_group(g):
            i0 = g * GRP
            xt = xpool.tile([H, GRP * WD], f32)
            # one dma_start per group: issue costs ~870ns of engine time
            # regardless of descriptor count
            nc.sync.dma_start(
                xt[:].rearrange("h (i w) -> h i w", i=GRP),
                x_flat[i0 : i0 + GRP, :, :].rearrange("i h w -> h i w"),
            )
            return xt

        def silu_group(xt):
            st = spool.tile([H, GRP * WD], bf16, tag="st")
            if native_silu:
                nc.scalar.activation(
                    st[:], xt[:], mybir.ActivationFunctionType.Silu
                )
            else:
                sg = spool.tile([H, GRP * WD], f32, tag="sg")
                nc.scalar.activation(
                    sg[:], xt[:], mybir.ActivationFunctionType.Sigmoid
                )
                nc.vector.tensor_mul(st[:], xt[:], sg[:])
            return st

        # software-pipeline silu one group ahead so it is issued on ACT's
        # queue BEFORE group g's drains (else silu(g+1) waits behind them
        # and the PE stalls at group boundaries)
        xt = load_group(0)
        st_next = silu_group(xt)
        for g in range(NGRP):
            st3 = st_next[:].rearrange("h (i w) -> h i w", i=GRP)
            if g + 1 < NGRP:
                xt = load_group(g + 1)
                st_next = silu_group(xt)

            # free layout (k, i, w): each k-half is a contiguous 4032B run
            # per partition, so the store can be split into 2 chunks that
            # keep large descriptors
            ot = opool.tile([HO, FREE], bf16)
            ot4 = ot[:].rearrange("p (k i w) -> p k i w", i=GRP, k=NK)
            for k in range(NK):
                ps = ppool.tile([HO, GRP * WO], f32)
                ps3 = ps[:].rearrange("p (i n) -> p i n", i=GRP)
                for dw in range(3):
                    nc.tensor.matmul(
                        ps3,
                        band4[:, k, dw, :],
                        st3[:, :, dw : dw + WO],
                        start=(dw == 0),
                        stop=(dw == 2),
                        perf_mode=mybir.MatmulPerfMode.DoublePixel,
                    )
                # drain PSUM -> bf16 SBUF: DVE takes 6 of 8, ACT (which
                # also runs silu) takes 2
                if k % 4 == 3:
                    nc.scalar.activation(
                        ot4[:, k, :, :], ps3, mybir.ActivationFunctionType.Copy
                    )
                else:
                    nc.vector.tensor_copy(ot4[:, k, :, :], ps3)
                # store in 2 chunks per group so the second chunk's DMA
                # overlaps the last drains (shrinks the tail)
                if k == 3:
                    nc.sync.dma_start(
                        out_d.ap()[g][:, 0 : FREE // 2], ot[:, 0 : FREE // 2]
                    )
                elif k == 7:
                    nc.sync.dma_start(
                        out_d.ap()[g][:, FREE // 2 : FREE], ot[:, FREE // 2 : FREE]
                    )

    nc.compile()
    return nc


def _get_module():
    if "nc" not in _CACHE:
        _CACHE["nc"] = _build_module()
    return _CACHE["nc"]


def _reassemble_core(dev_out: np.ndarray) -> np.ndarray:
    """Device layout [NGRP, HO, (k i w)] -> [B_LOC, C*NK, HO, WO] float32."""
    a = np.asarray(dev_out).reshape(NGRP, HO, NK, GRP, WO)
    a = a.transpose(0, 3, 2, 1, 4)  # [g, i, k, ho, wo]
    a = a.reshape(B_LOC, C * NK, HO, WO)
    return a.astype(np.float32)


def _postprocess_sim(out: np.ndarray) -> np.ndarray:
    return _reassemble_core(out)


def prepare(inputs):
    """Shard FULL inputs into per-core in_maps; return (in_maps, assemble)."""
    import ml_dtypes

    x = np.ascontiguousarray(np.asarray(inputs["x"], dtype=np.float32))
    W = np.asarray(inputs["W"], dtype=np.float32)
    assert x.shape == (B, C, H, WD), x.shape
    assert W.shape == (NK, 9), W.shape

    bands = _make_bands(W).astype(ml_dtypes.bfloat16)
    in_maps = [
        {"x": x[i * B_LOC : (i + 1) * B_LOC], "bands": bands} for i in range(NCORES)
    ]

    def assemble(results):
        from concurrent.futures import ThreadPoolExecutor

        with ThreadPoolExecutor(NCORES) as ex:
            parts = list(
                ex.map(lambda i: _reassemble_core(results[i]["out"]), range(NCORES))
            )
        return np.concatenate(parts, axis=0)

    return in_maps, assemble


def kernel(x: np.ndarray, W: np.ndarray) -> np.ndarray:
    from concourse.bass_utils import run_bass_kernel_spmd

    in_maps, assemble = prepare({"x": x, "W": W})
    nc = _get_module()
    res = run_bass_kernel_spmd(nc, in_maps, core_ids=list(range(NCORES)))
    return assemble(res.results)


# revision 16
# speedup vs baseline: 1.0090x; 1.0090x over previous
"""Trainium2 Bass kernel for nn_Conv2d_85830626443584.

Math (from the reference):
  x: [16, 64, 128, 128] f32, W: [8, 9] f32
  s = silu(x)
  out[b, c*8+k, ho, wo] = sum_{dh,dw} W[k, 3*dh+dw] * s[b, c, ho+dh, wo+dw]
  out: [16, 512, 126, 126] f32

Strategy (per NeuronCore, batch-sharded 16/8 = 2 batches -> 128 channel-images):
  * Each channel-image is an independent [128, 128] tile, SBUF layout
    [partition=h, free=w].
  * The 3x3 conv is computed as 3 PSUM-accumulating matmuls per output map k:
    a banded stationary matrix Band[(h_in=128), (ho=126)] carries the 3
    vertical taps (dh), and the horizontal taps (dw) come for free as
    rhs access-pattern column offsets:
       psum_k[ho, n] += sum_h Band_{k,dw}[h, ho] * s[h, n+dw]   (dw = 0,1,2)
    No im2col, no data duplication.
  * bf16 band + bf16 silu: matmul streams 1 col/cycle and the bf16 weight
    load (vs 4-byte f32r) hides under the 504-col stream.
  * Output is stored in bf16 in a DMA-friendly device layout
    [group, ho, (img, k, wo)] -- per-partition contiguous 8KB descriptors
    instead of 504B rows -- and reassembled + upcast to f32 on the host.
  * Images are processed in groups of 4 (rhs N = 4*126 = 504 <= 512 psum bank).
"""

import numpy as np

B, C, H, WD = 16, 64, 128, 128
NK = 8            # n_convs
HO = WO = 126     # output spatial dims
NCORES = 8
B_LOC = B // NCORES              # 2 batches per core
NIMG_LOC = B_LOC * C             # 128 images per core
GRP = 4                          # images per group
NGRP = NIMG_LOC // GRP           # 32 groups
FREE = GRP * NK * WO             # 4032 output elems per partition per group

_CACHE = {}


def _make_bands(W: np.ndarray) -> np.ndarray:
    """Banded stationary matrices, one [128, 126] per (k, dw).

    bands[h, k, dw, ho] = W[k, 3*dh + dw] where dh = h - ho in {0,1,2}.
    Returned flattened to [128, 8*3*126], float32 (caller casts to bf16).
    """
    bands = np.zeros((H, NK, 3, HO), dtype=np.float32)
    ho = np.arange(HO)
    for dh in range(3):
        for dw in range(3):
            bands[ho + dh, :, dw, ho] = W[:, 3 * dh + dw][None, :]
    return bands.reshape(H, NK * 3 * HO)


def _build_module(native_silu: bool = True):
    import concourse.mybir as mybir
    import concourse.tile as tile
    from concourse import bacc
    from contextlib import ExitStack

    f32 = mybir.dt.float32
    bf16 = mybir.dt.bfloat16

    nc = bacc.Bacc("TRN2", target_bir_lowering=False, debug=False)

    x_d = nc.dram_tensor("x", [B_LOC, C, H, WD], f32, kind="ExternalInput")
    bands_d = nc.dram_tensor("bands", [H, NK * 3 * HO], bf16, kind="ExternalInput")
    out_d = nc.dram_tensor("out", [NGRP, HO, FREE], bf16, kind="ExternalOutput")

    with tile.TileContext(nc) as tc, ExitStack() as ctx:
        cpool = ctx.enter_context(tc.tile_pool(name="const", bufs=1))
        xpool = ctx.enter_context(tc.tile_pool(name="xin", bufs=3))
        spool = ctx.enter_context(tc.tile_pool(name="silu", bufs=3))
        opool = ctx.enter_context(tc.tile_pool(name="outs", bufs=3))
        ppool = ctx.enter_context(tc.tile_pool(name="psum", bufs=8, space="PSUM"))

        band_t = cpool.tile([H, NK * 3 * HO], bf16)
        # band on scalar's queue so sync can issue group-0's load at once
        nc.scalar.dma_start(band_t[:], bands_d.ap())
        band4 = band_t[:].rearrange("p (k d m) -> p k d m", k=NK, d=3)

        # [128 images, 128 h, 128 w] view of the local input
        x_flat = x_d.ap().rearrange("b c h w -> (b c) h w")

        def load_group(g):
            i0 = g * GRP
            xt = xpool.tile([H, GRP * WD], f32)
            # one dma_start per group: issue costs ~870ns of engine time
            # regardless of descriptor count
            nc.sync.dma_start(
                xt[:].rearrange("h (i w) -> h i w", i=GRP),
                x_flat[i0 : i0 + GRP, :, :].rearrange("i h w -> h i w"),
            )
            return xt

        def silu_group(xt):
            st = spool.tile([H, GRP * WD], bf16, tag="st")
            if native_silu:
                nc.scalar.activation(
                    st[:], xt[:], mybir.ActivationFunctionType.Silu
                )
            else:
                sg = spool.tile([H, GRP * WD], f32, tag="sg")
                nc.scalar.activation(
                    sg[:], xt[:], mybir.ActivationFunctionType.Sigmoid
                )
                nc.vector.tensor_mul(st[:], xt[:], sg[:])
            return st

        # software-pipeline silu one group ahead so it is issued on ACT's
        # queue BEFORE group g's drains (else silu(g+1) waits behind them
        # and the PE stalls at group boundaries)
        xt = load_group(0)
        st_next = silu_group(xt)
        for g in range(NGRP):
            st3 = st_next[:].rearrange("h (i w) -> h i w", i=GRP)
            if g + 1 < NGRP:
                xt = load_group(g + 1)
                st_next = silu_group(xt)

            # free layout (k, i, w): each k-half is a contiguous 4032B run
            # per partition, so the store can be split into 2 chunks that
            # keep large descriptors
            ot = opool.tile([HO, FREE], bf16)
            ot4 = ot[:].rearrange("p (k i w) -> p k i w", i=GRP, k=NK)
            for k in range(NK):
                ps = ppool.tile([HO, GRP * WO], f32)
                ps3 = ps[:].rearrange("p (i n) -> p i n", i=GRP)
                for dw in range(3):
                    nc.tensor.matmul(
                        ps3,
                        band4[:, k, dw, :],
                        st3[:, :, dw : dw + WO],
                        start=(dw == 0),
                        stop=(dw == 2),
                    )
                # drain PSUM -> bf16 SBUF: DVE takes 6 of 8, ACT (which
                # also runs silu) takes 2
                if k % 4 == 3:
                    nc.scalar.activation(
                        ot4[:, k, :, :], ps3, mybir.ActivationFunctionType.Copy
                    )
                else:
                    nc.vector.tensor_copy(ot4[:, k, :, :], ps3)
                # store in 2 chunks per group so the second chunk's DMA
                # overlaps the last drains (shrinks the tail)
                if k == 3:
                    nc.sync.dma_start(
                        out_d.ap()[g][:, 0 : FREE // 2], ot[:, 0 : FREE // 2]
                    )
                elif k == 7:
                    nc.sync.dma_start(
                        out_d.ap()[g][:, FREE // 2 : FREE], ot[:, FREE // 2 : FREE]
                    )

    nc.compile()
    return nc


def _get_module():
    if "nc" not in _CACHE:
        _CACHE["nc"] = _build_module()
    return _CACHE["nc"]


def _reassemble_core(dev_out: np.ndarray) -> np.ndarray:
    """Device layout [NGRP, HO, (k i w)] -> [B_LOC, C*NK, HO, WO] float32."""
    a = np.asarray(dev_out).reshape(NGRP, HO, NK, GRP, WO)
    a = a.transpose(0, 3, 2, 1, 4)  # [g, i, k, ho, wo]
    a = a.reshape(B_LOC, C * NK, HO, WO)
    return a.astype(np.float32)


def _postprocess_sim(out: np.ndarray) -> np.ndarray:
    return _reassemble_core(out)


def prepare(inputs):
    """Shard FULL inputs into per-core in_maps; return (in_maps, assemble)."""
    import ml_dtypes

    x = np.ascontiguousarray(np.asarray(inputs["x"], dtype=np.float32))
    W = np.asarray(inputs["W"], dtype=np.float32)
    assert x.shape == (B, C, H, WD), x.shape
    assert W.shape == (NK, 9), W.shape

    bands = _make_bands(W).astype(ml_dtypes.bfloat16)
    in_maps = [
        {"x": x[i * B_LOC : (i + 1) * B_LOC], "bands": bands} for i in range(NCORES)
    ]

    def assemble(results):
        from concurrent.futures import ThreadPoolExecutor

        with ThreadPoolExecutor(NCORES) as ex:
            parts = list(
                ex.map(lambda i: _reassemble_core(results[i]["out"]), range(NCORES))
            )
        return np.concatenate(parts, axis=0)

    return in_maps, assemble


def kernel(x: np.ndarray, W: np.ndarray) -> np.ndarray:
    from concourse.bass_utils import run_bass_kernel_spmd

    in_maps, assemble = prepare({"x": x, "W": W})
    nc = _get_module()
    res = run_bass_kernel_spmd(nc, in_maps, core_ids=list(range(NCORES)))
    return assemble(res.results)
